# revision 31
# baseline (speedup 1.0000x reference)
"""Trainium2 Bass kernel for the CNODE (HiPPO continuous-time ODE) model.

Strategy (8 NeuronCores, pure data parallel over batch B=256 -> 32/core):
  - All state kept feature-major in SBUF: partition = feature (mod 128),
    free dim = (feature_block, batch).
  - MLP layers are weight-stationary matmuls: lhsT = weight tile [K,128],
    rhs = activations [K, 32].  No transposes anywhere.
  - cn0 (HiPPO coefficients) update: one block-diagonal matmul (kron(I4, A'^T))
    over all 4 feature blocks at once (N=128) plus 4 tiny C-matmuls injecting
    the Bv * u forcing term.  DT is folded into A/Bv/W3/b3 on the host.
  - cn1 state feature order is permuted to [psi(512); y(16)] so the obs-step
    reset cn1 <- [y_t, cn0] is partition-aligned with cn0 (pure elementwise).
  - Sequential time loop (T=50 obs x 5 euler) fully unrolled.
"""

import os
from contextlib import ExitStack

import numpy as np
import ml_dtypes

Nc, ID, HID = 32, 16, 512
DT, N_STEPS, B, T = 0.05, 5, 256, 50
D0 = ID * Nc            # 512
DIN = D0 + ID           # 528
NCORES = 8
BL = B // NCORES        # 32 batch per core
DELTA = 5.0

USE_BF16 = os.environ.get("CNODE_BF16", "0") == "1"
USE_TELE = os.environ.get("CNODE_TELE", "1") == "1"
T_RUN = int(os.environ.get("CNODE_T_RUN", str(T)))  # dev knob; harness uses 50

_CACHE: dict = {}
LAST_RESULT = None


# ---------------------------------------------------------------- program ---
def _build(bf16: bool, t_run: int, tele: bool, reps: int = 1):
    import concourse.bass as bass
    from concourse import bacc, mybir, tile

    f32 = mybir.dt.float32
    dtw = mybir.dt.bfloat16 if bf16 else f32
    ADD = mybir.AluOpType.add
    SUB = mybir.AluOpType.subtract
    MUL = mybir.AluOpType.mult
    MAX = mybir.AluOpType.max

    nc = bacc.Bacc("TRN2", target_bir_lowering=False, debug=False,
                   num_devices=NCORES)

    w1 = nc.dram_tensor("w1", [128, 5 * 512], dtw, kind="ExternalInput").ap()
    w2 = nc.dram_tensor("w2", [128, 4 * 512], dtw, kind="ExternalInput").ap()
    w3 = nc.dram_tensor("w3", [128, 4 * 528], dtw, kind="ExternalInput").ap()
    bd = nc.dram_tensor("bd", [128, 128], f32, kind="ExternalInput").ap()
    cu = nc.dram_tensor("cu", [16 * N_STEPS, 512], f32, kind="ExternalInput").ap()
    if tele:
        w31 = nc.dram_tensor("w31", [128, 4 * 512], dtw,
                             kind="ExternalInput").ap()
    b1 = nc.dram_tensor("b1", [128, 4 * N_STEPS], f32,
                        kind="ExternalInput").ap()
    b2 = nc.dram_tensor("b2", [128, 4], f32, kind="ExternalInput").ap()
    b3 = nc.dram_tensor("b3", [128, 5], f32, kind="ExternalInput").ap()
    yt = nc.dram_tensor("yt", [16, 32 * t_run], f32, kind="ExternalInput").ap()
    mt = nc.dram_tensor("mt", [128, 128 * t_run], f32, kind="ExternalInput").ap()

    preds = nc.dram_tensor("preds", [16, 32 * t_run], f32, kind="ExternalOutput").ap()
    traj0 = nc.dram_tensor("traj0", [t_run, 128, 128], f32, kind="ExternalOutput").ap()
    fin1 = nc.dram_tensor("fin1", [128, 160], f32, kind="ExternalOutput").ap()

    with tile.TileContext(nc) as tc, ExitStack() as ctx:
        wp = ctx.enter_context(tc.tile_pool(name="weights", bufs=1))
        sp = ctx.enter_context(tc.tile_pool(name="state", bufs=1))
        ap_ = ctx.enter_context(tc.tile_pool(name="acts", bufs=2))
        pp = ctx.enter_context(tc.tile_pool(name="psum", bufs=2, space="PSUM"))

        W1t = wp.tile([128, 5 * 512], dtw)
        W2t = wp.tile([128, 4 * 512], dtw)
        W3t = wp.tile([128, 4 * 528], dtw)
        if tele:
            W31t = wp.tile([128, 4 * 512], dtw)
        BDt = wp.tile([128, 128], f32)
        CUt = wp.tile([16 * N_STEPS, 512], f32)
        b1t = wp.tile([128, 4 * N_STEPS], f32)
        b2t = wp.tile([128, 4], f32)
        b3t = wp.tile([128, 5], f32)
        Yt = wp.tile([16, 32 * t_run], f32)
        Mt = wp.tile([128, 128 * t_run], f32)
        predt = wp.tile([16, 32 * t_run], f32)

        dma_pairs = [(W1t, w1), (W2t, w2), (W3t, w3), (BDt, bd), (CUt, cu),
                     (b1t, b1), (b2t, b2), (b3t, b3), (Yt, yt), (Mt, mt)]
        if tele:
            dma_pairs.append((W31t, w31))
        for dst, src in dma_pairs:
            nc.sync.dma_start(dst[:], src[:])

        cn0 = sp.tile([128, 128], f32)
        cn1 = sp.tile([128, 160], f32)
        Ut = sp.tile([16 * N_STEPS, 32], f32)
        if bf16:
            cn1x = sp.tile([128, 160], dtw)
        else:
            cn1x = cn1

        def x_slice(k):
            # MLP input K-tile k (feature-major): psi blocks 0..3, then y.
            if k < 4:
                return cn1x[:, 32 * k:32 * k + 32]
            return cn1x[0:16, 128:160]

        def euler_step(p1, s, h2_prev):
            p2 = pp.tile([128, 128], f32, tag="p2")
            p3 = pp.tile([128, 160], f32, tag="p3")
            h1 = ap_.tile([128, 128], dtw, tag="h1")
            h2 = ap_.tile([128, 128], dtw, tag="h2")

            # capture u(s) = current y-part into the stacked-U tile (feeds the
            # folded cn0 update at interval end)
            nc.sync.dma_start(Ut[16 * s:16 * s + 16, :], cn1[0:16, 128:160])

            # L1: h1 = relu(L1psum + b1 + s*c0)
            # s==0: L1psum = W1p^T x (fresh, 528-contraction)
            # s>0 (tele): L1psum += W31^T h2_prev  (accumulate in PSUM)
            if tele and s > 0:
                for m in range(4):
                    for k in range(4):
                        nc.tensor.matmul(
                            p1[:, 32 * m:32 * m + 32],
                            W31t[:, k * 512 + m * 128:k * 512 + m * 128 + 128],
                            h2_prev[:, 32 * k:32 * k + 32],
                            start=False, stop=(k == 3), skip_group_check=True)
            else:
                for m in range(4):
                    for k in range(5):
                        kk = 128 if k < 4 else 16
                        # start=True clears has_written for the WHOLE bank, so
                        # in tele mode only the first MM of the interval may
                        # set it (later steps accumulate onto every m-tile).
                        st = (m == 0 and k == 0) if tele else (k == 0)
                        nc.tensor.matmul(
                            p1[:, 32 * m:32 * m + 32],
                            W1t[0:kk, k * 512 + m * 128:k * 512 + m * 128 + 128],
                            x_slice(k), start=st, stop=(k == 4),
                            skip_group_check=tele)
            bs = (s * 4) if tele else 0
            for m in range(4):
                nc.vector.tensor_scalar(
                    h1[:, 32 * m:32 * m + 32], p1[:, 32 * m:32 * m + 32],
                    b1t[:, bs + m:bs + m + 1], 0.0, ADD, MAX)

            # L2: h2 = relu(W2^T h1 + b2)   (k-outer: MM (m,k) only needs h1
            # block k, so the PE never waits on the last relu of L1)
            for k in range(4):
                for m in range(4):
                    nc.tensor.matmul(
                        p2[:, 32 * m:32 * m + 32],
                        W2t[:, k * 512 + m * 128:k * 512 + m * 128 + 128],
                        h1[:, 32 * k:32 * k + 32],
                        start=(k == 0 and m == 0), stop=(k == 3),
                        skip_group_check=True)
            for m in range(4):
                nc.vector.tensor_scalar(
                    h2[:, 32 * m:32 * m + 32], p2[:, 32 * m:32 * m + 32],
                    b2t[:, m:m + 1], 0.0, ADD, MAX)

            # L3: p3 = W3p'^T h2   (DT and output permutation folded in)
            for k in range(4):
                for m in range(5):
                    mm = 128 if m < 4 else 16
                    oc = (slice(0, 128), slice(32 * m, 32 * m + 32)) if m < 4 \
                        else (slice(0, 16), slice(128, 160))
                    nc.tensor.matmul(
                        p3[oc[0], oc[1]],
                        W3t[:, k * 528 + m * 128:k * 528 + m * 128 + mm],
                        h2[:, 32 * k:32 * k + 32],
                        start=(k == 0 and m == 0), stop=(k == 3),
                        skip_group_check=True)

            # cn1 += p3 + b3'   (bf16 shadow first: it gates next step's L1)
            for m in range(5):
                if m < 4:
                    po, co = slice(0, 128), slice(32 * m, 32 * m + 32)
                    bcol = b3t[:, m:m + 1]
                else:
                    po, co = slice(0, 16), slice(128, 160)
                    bcol = b3t[0:16, 4:5]
                if bf16 and not tele:
                    nc.vector.scalar_tensor_tensor(
                        cn1x[po, co], p3[po, co], bcol, cn1[po, co], ADD, ADD)
                nc.vector.scalar_tensor_tensor(
                    cn1[po, co], p3[po, co], bcol, cn1[po, co], ADD, ADD)
            return h2

        def cn0_interval_update():
            # cn0 <- G5-blockdiag^T cn0 + CU^T U   (whole obs interval folded)
            pc = pp.tile([128, 128], f32, tag="pc")
            nc.tensor.matmul(pc[:, 0:128], BDt[:, :], cn0[:, 0:128],
                             start=True, stop=False)
            for d in range(4):
                nc.tensor.matmul(pc[:, 32 * d:32 * d + 32],
                                 CUt[0:80, 128 * d:128 * d + 128],
                                 Ut[0:80, :],
                                 start=False, stop=(d == 3),
                                 skip_group_check=True)
            nc.vector.tensor_copy(cn0[:, :], pc[:, 0:128])

        def obs_update(t):
            # prediction = y-part of h_end (pre-mix)
            nc.vector.tensor_copy(predt[:, 32 * t:32 * t + 32],
                                  cn1[0:16, 128:160])
            # cn1 <- cn1 + m * (upd - cn1); upd = [cn0 (psi); y_t]
            t1 = ap_.tile([128, 128], f32, tag="mix1")
            t2 = ap_.tile([128, 128], f32, tag="mix2")
            nc.vector.tensor_tensor(t1[:, :], cn0[:, :], cn1[:, 0:128], SUB)
            nc.vector.tensor_tensor(t2[:, :], t1[:, :],
                                    Mt[:, 128 * t:128 * t + 128], MUL)
            if bf16:
                nc.vector.tensor_tensor(cn1x[:, 0:128], t2[:, :],
                                        cn1[:, 0:128], ADD)
            nc.vector.tensor_tensor(cn1[:, 0:128], t2[:, :], cn1[:, 0:128], ADD)

            t1y = ap_.tile([16, 32], f32, tag="mix1y")
            t2y = ap_.tile([16, 32], f32, tag="mix2y")
            nc.vector.tensor_tensor(t1y[:, :], Yt[0:16, 32 * t:32 * t + 32],
                                    cn1[0:16, 128:160], SUB)
            nc.vector.tensor_tensor(t2y[:, :], t1y[:, :],
                                    Mt[0:16, 128 * t:128 * t + 32], MUL)
            if bf16:
                nc.vector.tensor_tensor(cn1x[0:16, 128:160], t2y[:, :],
                                        cn1[0:16, 128:160], ADD)
            nc.vector.tensor_tensor(cn1[0:16, 128:160], t2y[:, :],
                                    cn1[0:16, 128:160], ADD)

            nc.sync.dma_start(traj0[t], cn0[:, :])
            if t == t_run - 1:
                nc.sync.dma_start(fin1[:], cn1[:, :])

        for _rep in range(reps):
            nc.vector.memset(cn0[:], 0.0)
            nc.vector.memset(cn1[:], 0.0)
            if bf16:
                nc.vector.memset(cn1x[:], 0.0)
            for t in range(t_run):
                p1 = pp.tile([128, 128], f32, tag="p1")
                h2_prev = None
                for s in range(N_STEPS):
                    if not tele and s > 0:
                        p1 = pp.tile([128, 128], f32, tag="p1")
                    h2_prev = euler_step(p1, s, h2_prev)
                cn0_interval_update()
                obs_update(t)

        nc.sync.dma_start(preds[:], predt[:])

    nc.compile()
    return nc


# ------------------------------------------------------------- host packing ---
def _pack_shared(inputs, bf16: bool, tele: bool):
    f32 = np.float32
    wdt = ml_dtypes.bfloat16 if bf16 else f32
    W1 = np.asarray(inputs["W1"], f32)
    W2 = np.asarray(inputs["W2"], f32)
    W3 = np.asarray(inputs["W3"], f32)
    b1v = np.asarray(inputs["b1"], f32)
    b2v = np.asarray(inputs["b2"], f32)
    b3v = np.asarray(inputs["b3"], f32)
    A = np.asarray(inputs["A"], f32)
    Bv = np.asarray(inputs["Bv"], f32)

    perm = np.r_[ID:DIN, 0:ID]             # cn1 features -> [psi; y]
    W1p = W1[perm, :]                      # [528, 512]
    W3p = (DT * W3)[:, perm]               # [512, 528]
    b3p = (DT * b3v)[perm]

    w1sb = np.zeros((128, 5 * 512), f32)
    for k in range(5):
        kk = 128 if k < 4 else 16
        w1sb[0:kk, k * 512:(k + 1) * 512] = W1p[k * 128:k * 128 + kk, :]
    w2sb = np.zeros((128, 4 * 512), f32)
    for k in range(4):
        w2sb[:, k * 512:(k + 1) * 512] = W2[k * 128:(k + 1) * 128, :]
    w3sb = np.zeros((128, 4 * 528), f32)
    for k in range(4):
        w3sb[:, k * 528:(k + 1) * 528] = W3p[k * 128:(k + 1) * 128, :]

    # Folded per-interval cn0 map: c(5) = G^5 c(0) + sum_s G^(4-s) DT*Bv u(s)
    G = np.eye(Nc, dtype=np.float64) + np.float64(DT) * A.astype(np.float64)
    G5 = np.linalg.matrix_power(G, N_STEPS)
    bdsb = np.kron(np.eye(4), G5.T).astype(f32)                    # [128,128]
    dtbv = np.float64(DT) * Bv.astype(np.float64)
    cusb = np.zeros((16 * N_STEPS, 512), np.float64)
    for s in range(N_STEPS):
        v = np.linalg.matrix_power(G, N_STEPS - 1 - s) @ dtbv      # [32]
        for i in range(16):
            base = 128 * (i // 4) + 32 * (i % 4)
            cusb[16 * s + i, base:base + 32] = v
    cusb = cusb.astype(f32)

    b2sb = b2v.reshape(4, 128).T.copy()
    b3sb = np.zeros((128, 5), f32)
    b3sb[:, 0:4] = b3p[0:512].reshape(4, 128).T
    b3sb[0:16, 4] = b3p[512:528]

    c0 = DT * (W1.T @ b3v)                 # telescoped bias increment [512]
    b1sb = np.zeros((128, 4 * N_STEPS), f32)
    for s in range(N_STEPS):
        b1sb[:, 4 * s:4 * s + 4] = (b1v + s * c0).reshape(4, 128).T

    out = {"w1": w1sb.astype(wdt), "w2": w2sb.astype(wdt),
           "w3": w3sb.astype(wdt), "bd": bdsb, "cu": cusb,
           "b1": b1sb, "b2": b2sb, "b3": b3sb}
    if tele:
        W31 = (DT * (W3 @ W1)).astype(f32)  # [512, 512]
        w31sb = np.zeros((128, 4 * 512), f32)
        for k in range(4):
            w31sb[:, k * 512:(k + 1) * 512] = W31[k * 128:(k + 1) * 128, :]
        out["w31"] = w31sb.astype(wdt)
    return out


def _pack_core(inputs, c: int, t_run: int):
    f32 = np.float32
    Y = np.asarray(inputs["Y"], f32)[c * BL:(c + 1) * BL]       # [32, T, 16]
    mask = np.asarray(inputs["mask"], f32)[c * BL:(c + 1) * BL]  # [32, T]
    ysb = Y[:, 0:t_run, :].transpose(2, 1, 0).reshape(16, t_run * 32).copy()
    mtb = mask[:, 0:t_run].T                                    # [t, b]
    mrow = np.tile(mtb, (1, 4)).reshape(1, t_run * 128)
    mtsb = np.broadcast_to(mrow, (128, t_run * 128)).copy()
    return {"yt": ysb, "mt": mtsb}


def kernel(**inputs):
    from concourse.bass_utils import run_bass_kernel_spmd

    bf16, t_run, tele = USE_BF16, T_RUN, USE_TELE
    key = (bf16, t_run, tele)
    if key not in _CACHE:
        _CACHE[key] = _build(bf16, t_run, tele)
    nc = _CACHE[key]

    shared = _pack_shared(inputs, bf16, tele)
    in_maps = [dict(shared, **_pack_core(inputs, c, t_run))
               for c in range(NCORES)]

    res = run_bass_kernel_spmd(nc, in_maps, core_ids=list(range(NCORES)))
    global LAST_RESULT
    LAST_RESULT = res
    outs = res.results

    f32 = np.float32
    mask = np.asarray(inputs["mask"], f32)[:, 0:t_run]
    y_preds = np.zeros((B, t_run, ID), f32)
    h_fin = np.zeros((B, 2 * D0 + ID), f32)
    cn0_traj = np.zeros((t_run, B, D0), f32)
    for c in range(NCORES):
        o = outs[c]
        sl = slice(c * BL, (c + 1) * BL)
        y_preds[sl] = np.asarray(o["preds"], f32).reshape(
            16, t_run, 32).transpose(2, 1, 0)
        tr = np.asarray(o["traj0"], f32).reshape(
            t_run, 128, 4, 32).transpose(0, 3, 2, 1).reshape(t_run, 32, D0)
        cn0_traj[:, sl, :] = tr
        f1 = np.asarray(o["fin1"], f32)
        psi = f1[:, 0:128].reshape(128, 4, 32).transpose(2, 1, 0).reshape(32, D0)
        yfin = f1[0:16, 128:160].T
        h_fin[sl, 0:D0] = tr[t_run - 1]
        h_fin[sl, D0:D0 + ID] = yfin
        h_fin[sl, D0 + ID:] = psi

    any_t = np.nonzero(mask.any(axis=0))[0]
    if len(any_t):
        last_h_cn = cn0_traj[any_t.max()].copy()
    else:
        last_h_cn = np.zeros((B, D0), f32)

    times = np.asarray(inputs["times"], f32)
    return y_preds, y_preds.copy(), times, last_h_cn, h_fin


# revision 32
# speedup vs baseline: 1.0207x; 1.0207x over previous
"""Trainium2 Bass kernel for the CNODE (HiPPO continuous-time ODE) model.

Strategy (8 NeuronCores, pure data parallel over batch B=256 -> 32/core):
  - All state kept feature-major in SBUF: partition = feature (mod 128),
    free dim = (feature_block, batch).
  - MLP layers are weight-stationary matmuls: lhsT = weight tile [K,128],
    rhs = activations [K, 32].  No transposes anywhere.
  - cn0 (HiPPO coefficients) update folded per obs interval on the host:
    c(5) = G^5 c(0) + sum_s G^(4-s)*DT*Bv u(s) with G = I + DT*A, so the
    5 euler substeps cost one kron(I4, G5^T) matmul (N=128) plus 4 stacked
    forcing matmuls (K=80) per interval.  This path stays fp32 (bf16 here
    is catastrophically unstable).
  - Telescoped L1: within an interval, L1psum(s+1) = L1psum(s) + W31^T h2(s)
    + s*c0 accumulated in PSUM (W31 = DT*W3@W1), so only the first substep
    pays the 528-contraction and the PE never waits on a DVE round trip at
    step boundaries.
  - cn1 state feature order is permuted to [psi(512); y(16)] so the obs-step
    reset cn1 <- [y_t, cn0] is partition-aligned with cn0 (pure elementwise).
  - Sequential time loop (T=50 obs x 5 euler) fully unrolled.
"""

import os
from contextlib import ExitStack

import numpy as np
import ml_dtypes

Nc, ID, HID = 32, 16, 512
DT, N_STEPS, B, T = 0.05, 5, 256, 50
D0 = ID * Nc            # 512
DIN = D0 + ID           # 528
NCORES = 8
BL = B // NCORES        # 32 batch per core
DELTA = 5.0

USE_BF16 = os.environ.get("CNODE_BF16", "0") == "1"
USE_TELE = os.environ.get("CNODE_TELE", "1") == "1"
T_RUN = int(os.environ.get("CNODE_T_RUN", str(T)))  # dev knob; harness uses 50

_CACHE: dict = {}
LAST_RESULT = None


# ---------------------------------------------------------------- program ---
def _build(bf16: bool, t_run: int, tele: bool, reps: int = 1):
    import concourse.bass as bass
    from concourse import bacc, mybir, tile

    f32 = mybir.dt.float32
    dtw = mybir.dt.bfloat16 if bf16 else f32
    ADD = mybir.AluOpType.add
    SUB = mybir.AluOpType.subtract
    MUL = mybir.AluOpType.mult
    MAX = mybir.AluOpType.max

    nc = bacc.Bacc("TRN2", target_bir_lowering=False, debug=False,
                   num_devices=NCORES)

    w1 = nc.dram_tensor("w1", [128, 5 * 512], dtw, kind="ExternalInput").ap()
    w2 = nc.dram_tensor("w2", [128, 4 * 512], dtw, kind="ExternalInput").ap()
    w3 = nc.dram_tensor("w3", [128, 4 * 528], dtw, kind="ExternalInput").ap()
    bd = nc.dram_tensor("bd", [128, 128], f32, kind="ExternalInput").ap()
    cu = nc.dram_tensor("cu", [16 * N_STEPS, 512], f32, kind="ExternalInput").ap()
    if tele:
        w31 = nc.dram_tensor("w31", [128, 4 * 512], dtw,
                             kind="ExternalInput").ap()
    b1 = nc.dram_tensor("b1", [128, 4 * N_STEPS], f32,
                        kind="ExternalInput").ap()
    b2 = nc.dram_tensor("b2", [128, 4], f32, kind="ExternalInput").ap()
    b3 = nc.dram_tensor("b3", [128, 5], f32, kind="ExternalInput").ap()
    yt = nc.dram_tensor("yt", [16, 32 * t_run], f32, kind="ExternalInput").ap()
    mt = nc.dram_tensor("mt", [128, 128 * t_run], f32, kind="ExternalInput").ap()

    preds = nc.dram_tensor("preds", [16, 32 * t_run], f32, kind="ExternalOutput").ap()
    traj0 = nc.dram_tensor("traj0", [t_run, 128, 128], f32, kind="ExternalOutput").ap()
    fin1 = nc.dram_tensor("fin1", [128, 160], f32, kind="ExternalOutput").ap()

    with tile.TileContext(nc) as tc, ExitStack() as ctx:
        wp = ctx.enter_context(tc.tile_pool(name="weights", bufs=1))
        sp = ctx.enter_context(tc.tile_pool(name="state", bufs=1))
        ap_ = ctx.enter_context(tc.tile_pool(name="acts", bufs=2))
        pp = ctx.enter_context(tc.tile_pool(name="psum", bufs=2, space="PSUM"))

        W1t = wp.tile([128, 5 * 512], dtw)
        W2t = wp.tile([128, 4 * 512], dtw)
        W3t = wp.tile([128, 4 * 528], dtw)
        if tele:
            W31t = wp.tile([128, 4 * 512], dtw)
        BDt = wp.tile([128, 128], f32)
        CUt = wp.tile([16 * N_STEPS, 512], f32)
        b1t = wp.tile([128, 4 * N_STEPS], f32)
        b2t = wp.tile([128, 4], f32)
        b3t = wp.tile([128, 5], f32)
        Yt = wp.tile([16, 32 * t_run], f32)
        Mt = wp.tile([128, 128 * t_run], f32)
        predt = wp.tile([16, 32 * t_run], f32)

        dma_pairs = [(W1t, w1), (W2t, w2), (W3t, w3), (BDt, bd), (CUt, cu),
                     (b1t, b1), (b2t, b2), (b3t, b3), (Yt, yt), (Mt, mt)]
        if tele:
            dma_pairs.append((W31t, w31))
        for dst, src in dma_pairs:
            nc.sync.dma_start(dst[:], src[:])

        cn0 = sp.tile([128, 128], f32)
        cn1 = sp.tile([128, 160], f32)
        Ut = sp.tile([16 * N_STEPS, 32], f32)
        if bf16:
            cn1x = sp.tile([128, 160], dtw)
        else:
            cn1x = cn1

        def x_slice(k):
            # MLP input K-tile k (feature-major): psi blocks 0..3, then y.
            if k < 4:
                return cn1x[:, 32 * k:32 * k + 32]
            return cn1x[0:16, 128:160]

        def euler_step(p1, s, h2_prev):
            p2 = pp.tile([128, 128], f32, tag="p2")
            p3 = pp.tile([128, 160], f32, tag="p3")
            h1 = ap_.tile([128, 128], dtw, tag="h1")
            h2 = ap_.tile([128, 128], dtw, tag="h2")

            # capture u(s) = current y-part into the stacked-U tile (feeds the
            # folded cn0 update at interval end)
            nc.sync.dma_start(Ut[16 * s:16 * s + 16, :], cn1[0:16, 128:160])

            # L1: h1 = relu(L1psum + b1 + s*c0)
            # s==0: L1psum = W1p^T x (fresh, 528-contraction)
            # s>0 (tele): L1psum += W31^T h2_prev  (accumulate in PSUM)
            if tele and s > 0:
                for m in range(4):
                    for k in range(4):
                        nc.tensor.matmul(
                            p1[:, 32 * m:32 * m + 32],
                            W31t[:, k * 512 + m * 128:k * 512 + m * 128 + 128],
                            h2_prev[:, 32 * k:32 * k + 32],
                            start=False, stop=(k == 3), skip_group_check=True)
            else:
                for m in range(4):
                    for k in range(5):
                        kk = 128 if k < 4 else 16
                        # start=True clears has_written for the WHOLE bank, so
                        # in tele mode only the first MM of the interval may
                        # set it (later steps accumulate onto every m-tile).
                        st = (m == 0 and k == 0) if tele else (k == 0)
                        nc.tensor.matmul(
                            p1[:, 32 * m:32 * m + 32],
                            W1t[0:kk, k * 512 + m * 128:k * 512 + m * 128 + 128],
                            x_slice(k), start=st, stop=(k == 4),
                            skip_group_check=tele)
            bs = (s * 4) if tele else 0
            for m in range(4):
                nc.vector.tensor_scalar(
                    h1[:, 32 * m:32 * m + 32], p1[:, 32 * m:32 * m + 32],
                    b1t[:, bs + m:bs + m + 1], 0.0, ADD, MAX)

            # L2: h2 = relu(W2^T h1 + b2)   (k-outer: MM (m,k) only needs h1
            # block k, so the PE never waits on the last relu of L1)
            for k in range(4):
                for m in range(4):
                    nc.tensor.matmul(
                        p2[:, 32 * m:32 * m + 32],
                        W2t[:, k * 512 + m * 128:k * 512 + m * 128 + 128],
                        h1[:, 32 * k:32 * k + 32],
                        start=(k == 0 and m == 0), stop=(k == 3),
                        skip_group_check=True)
            for m in range(4):
                nc.vector.tensor_scalar(
                    h2[:, 32 * m:32 * m + 32], p2[:, 32 * m:32 * m + 32],
                    b2t[:, m:m + 1], 0.0, ADD, MAX)

            # L3: p3 = W3p'^T h2   (DT and output permutation folded in)
            for k in range(4):
                for m in range(5):
                    mm = 128 if m < 4 else 16
                    oc = (slice(0, 128), slice(32 * m, 32 * m + 32)) if m < 4 \
                        else (slice(0, 16), slice(128, 160))
                    nc.tensor.matmul(
                        p3[oc[0], oc[1]],
                        W3t[:, k * 528 + m * 128:k * 528 + m * 128 + mm],
                        h2[:, 32 * k:32 * k + 32],
                        start=(k == 0 and m == 0), stop=(k == 3),
                        skip_group_check=True)

            # cn1 += p3 + b3'   (bf16 shadow first: it gates next step's L1)
            for m in range(5):
                if m < 4:
                    po, co = slice(0, 128), slice(32 * m, 32 * m + 32)
                    bcol = b3t[:, m:m + 1]
                else:
                    po, co = slice(0, 16), slice(128, 160)
                    bcol = b3t[0:16, 4:5]
                if bf16 and not tele:
                    nc.vector.scalar_tensor_tensor(
                        cn1x[po, co], p3[po, co], bcol, cn1[po, co], ADD, ADD)
                nc.vector.scalar_tensor_tensor(
                    cn1[po, co], p3[po, co], bcol, cn1[po, co], ADD, ADD)
            return h2

        def cn0_interval_update():
            # cn0 <- G5-blockdiag^T cn0 + CU^T U   (whole obs interval folded)
            pc = pp.tile([128, 128], f32, tag="pc")
            nc.tensor.matmul(pc[:, 0:128], BDt[:, :], cn0[:, 0:128],
                             start=True, stop=False)
            for d in range(4):
                nc.tensor.matmul(pc[:, 32 * d:32 * d + 32],
                                 CUt[0:80, 128 * d:128 * d + 128],
                                 Ut[0:80, :],
                                 start=False, stop=(d == 3),
                                 skip_group_check=True)
            nc.vector.tensor_copy(cn0[:, :], pc[:, 0:128])

        def obs_update(t):
            # prediction = y-part of h_end (pre-mix)
            nc.vector.tensor_copy(predt[:, 32 * t:32 * t + 32],
                                  cn1[0:16, 128:160])
            # cn1 <- cn1 + m * (upd - cn1); upd = [cn0 (psi); y_t]
            t1 = ap_.tile([128, 128], f32, tag="mix1")
            t2 = ap_.tile([128, 128], f32, tag="mix2")
            nc.vector.tensor_tensor(t1[:, :], cn0[:, :], cn1[:, 0:128], SUB)
            nc.vector.tensor_tensor(t2[:, :], t1[:, :],
                                    Mt[:, 128 * t:128 * t + 128], MUL)
            if bf16:
                nc.vector.tensor_tensor(cn1x[:, 0:128], t2[:, :],
                                        cn1[:, 0:128], ADD)
            nc.vector.tensor_tensor(cn1[:, 0:128], t2[:, :], cn1[:, 0:128], ADD)

            t1y = ap_.tile([16, 32], f32, tag="mix1y")
            t2y = ap_.tile([16, 32], f32, tag="mix2y")
            nc.vector.tensor_tensor(t1y[:, :], Yt[0:16, 32 * t:32 * t + 32],
                                    cn1[0:16, 128:160], SUB)
            nc.vector.tensor_tensor(t2y[:, :], t1y[:, :],
                                    Mt[0:16, 128 * t:128 * t + 32], MUL)
            if bf16:
                nc.vector.tensor_tensor(cn1x[0:16, 128:160], t2y[:, :],
                                        cn1[0:16, 128:160], ADD)
            nc.vector.tensor_tensor(cn1[0:16, 128:160], t2y[:, :],
                                    cn1[0:16, 128:160], ADD)

            nc.sync.dma_start(traj0[t], cn0[:, :])
            if t == t_run - 1:
                nc.sync.dma_start(fin1[:], cn1[:, :])

        for _rep in range(reps):
            nc.vector.memset(cn0[:], 0.0)
            nc.vector.memset(cn1[:], 0.0)
            if bf16:
                nc.vector.memset(cn1x[:], 0.0)
            for t in range(t_run):
                p1 = pp.tile([128, 128], f32, tag="p1")
                h2_prev = None
                for s in range(N_STEPS):
                    if not tele and s > 0:
                        p1 = pp.tile([128, 128], f32, tag="p1")
                    h2_prev = euler_step(p1, s, h2_prev)
                cn0_interval_update()
                obs_update(t)

        nc.sync.dma_start(preds[:], predt[:])

    nc.compile()
    return nc


# ------------------------------------------------------------- host packing ---
def _pack_shared(inputs, bf16: bool, tele: bool):
    f32 = np.float32
    wdt = ml_dtypes.bfloat16 if bf16 else f32
    W1 = np.asarray(inputs["W1"], f32)
    W2 = np.asarray(inputs["W2"], f32)
    W3 = np.asarray(inputs["W3"], f32)
    b1v = np.asarray(inputs["b1"], f32)
    b2v = np.asarray(inputs["b2"], f32)
    b3v = np.asarray(inputs["b3"], f32)
    A = np.asarray(inputs["A"], f32)
    Bv = np.asarray(inputs["Bv"], f32)

    perm = np.r_[ID:DIN, 0:ID]             # cn1 features -> [psi; y]
    W1p = W1[perm, :]                      # [528, 512]
    W3p = (DT * W3)[:, perm]               # [512, 528]
    b3p = (DT * b3v)[perm]

    w1sb = np.zeros((128, 5 * 512), f32)
    for k in range(5):
        kk = 128 if k < 4 else 16
        w1sb[0:kk, k * 512:(k + 1) * 512] = W1p[k * 128:k * 128 + kk, :]
    w2sb = np.zeros((128, 4 * 512), f32)
    for k in range(4):
        w2sb[:, k * 512:(k + 1) * 512] = W2[k * 128:(k + 1) * 128, :]
    w3sb = np.zeros((128, 4 * 528), f32)
    for k in range(4):
        w3sb[:, k * 528:(k + 1) * 528] = W3p[k * 128:(k + 1) * 128, :]

    # Folded per-interval cn0 map: c(5) = G^5 c(0) + sum_s G^(4-s) DT*Bv u(s)
    G = np.eye(Nc, dtype=np.float64) + np.float64(DT) * A.astype(np.float64)
    G5 = np.linalg.matrix_power(G, N_STEPS)
    bdsb = np.kron(np.eye(4), G5.T).astype(f32)                    # [128,128]
    dtbv = np.float64(DT) * Bv.astype(np.float64)
    cusb = np.zeros((16 * N_STEPS, 512), np.float64)
    for s in range(N_STEPS):
        v = np.linalg.matrix_power(G, N_STEPS - 1 - s) @ dtbv      # [32]
        for i in range(16):
            base = 128 * (i // 4) + 32 * (i % 4)
            cusb[16 * s + i, base:base + 32] = v
    cusb = cusb.astype(f32)

    b2sb = b2v.reshape(4, 128).T.copy()
    b3sb = np.zeros((128, 5), f32)
    b3sb[:, 0:4] = b3p[0:512].reshape(4, 128).T
    b3sb[0:16, 4] = b3p[512:528]

    c0 = DT * (W1.T @ b3v)                 # telescoped bias increment [512]
    b1sb = np.zeros((128, 4 * N_STEPS), f32)
    for s in range(N_STEPS):
        b1sb[:, 4 * s:4 * s + 4] = (b1v + s * c0).reshape(4, 128).T

    out = {"w1": w1sb.astype(wdt), "w2": w2sb.astype(wdt),
           "w3": w3sb.astype(wdt), "bd": bdsb, "cu": cusb,
           "b1": b1sb, "b2": b2sb, "b3": b3sb}
    if tele:
        W31 = (DT * (W3 @ W1)).astype(f32)  # [512, 512]
        w31sb = np.zeros((128, 4 * 512), f32)
        for k in range(4):
            w31sb[:, k * 512:(k + 1) * 512] = W31[k * 128:(k + 1) * 128, :]
        out["w31"] = w31sb.astype(wdt)
    return out


def _pack_core(inputs, c: int, t_run: int):
    f32 = np.float32
    Y = np.asarray(inputs["Y"], f32)[c * BL:(c + 1) * BL]       # [32, T, 16]
    mask = np.asarray(inputs["mask"], f32)[c * BL:(c + 1) * BL]  # [32, T]
    ysb = Y[:, 0:t_run, :].transpose(2, 1, 0).reshape(16, t_run * 32).copy()
    mtb = mask[:, 0:t_run].T                                    # [t, b]
    mrow = np.tile(mtb, (1, 4)).reshape(1, t_run * 128)
    mtsb = np.broadcast_to(mrow, (128, t_run * 128)).copy()
    return {"yt": ysb, "mt": mtsb}


def kernel(**inputs):
    from concourse.bass_utils import run_bass_kernel_spmd

    bf16, t_run, tele = USE_BF16, T_RUN, USE_TELE
    key = (bf16, t_run, tele)
    if key not in _CACHE:
        _CACHE[key] = _build(bf16, t_run, tele)
    nc = _CACHE[key]

    shared = _pack_shared(inputs, bf16, tele)
    in_maps = [dict(shared, **_pack_core(inputs, c, t_run))
               for c in range(NCORES)]

    res = run_bass_kernel_spmd(nc, in_maps, core_ids=list(range(NCORES)))
    global LAST_RESULT
    LAST_RESULT = res
    outs = res.results

    f32 = np.float32
    mask = np.asarray(inputs["mask"], f32)[:, 0:t_run]
    y_preds = np.zeros((B, t_run, ID), f32)
    h_fin = np.zeros((B, 2 * D0 + ID), f32)
    cn0_traj = np.zeros((t_run, B, D0), f32)
    for c in range(NCORES):
        o = outs[c]
        sl = slice(c * BL, (c + 1) * BL)
        y_preds[sl] = np.asarray(o["preds"], f32).reshape(
            16, t_run, 32).transpose(2, 1, 0)
        tr = np.asarray(o["traj0"], f32).reshape(
            t_run, 128, 4, 32).transpose(0, 3, 2, 1).reshape(t_run, 32, D0)
        cn0_traj[:, sl, :] = tr
        f1 = np.asarray(o["fin1"], f32)
        psi = f1[:, 0:128].reshape(128, 4, 32).transpose(2, 1, 0).reshape(32, D0)
        yfin = f1[0:16, 128:160].T
        h_fin[sl, 0:D0] = tr[t_run - 1]
        h_fin[sl, D0:D0 + ID] = yfin
        h_fin[sl, D0 + ID:] = psi

    any_t = np.nonzero(mask.any(axis=0))[0]
    if len(any_t):
        last_h_cn = cn0_traj[any_t.max()].copy()
    else:
        last_h_cn = np.zeros((B, D0), f32)

    times = np.asarray(inputs["times"], f32)
    return y_preds, y_preds.copy(), times, last_h_cn, h_fin


# revision 37
# speedup vs baseline: 1.0574x; 1.0360x over previous
"""Trainium2 Bass kernel for the CNODE (HiPPO continuous-time ODE) model.

Strategy (8 NeuronCores, pure data parallel over batch B=256 -> 32/core):
  - All state kept feature-major in SBUF: partition = feature (mod 128),
    free dim = (feature_block, batch).
  - MLP layers are weight-stationary matmuls: lhsT = weight tile [K,128],
    rhs = activations [K, 32].  No transposes anywhere.
  - cn0 (HiPPO coefficients) update folded per obs interval on the host:
    c(5) = G^5 c(0) + sum_s G^(4-s)*DT*Bv u(s) with G = I + DT*A, so the
    5 euler substeps cost one kron(I4, G5^T) matmul (N=128) plus 4 stacked
    forcing matmuls (K=80) per interval.  This path stays fp32 (bf16 here
    is catastrophically unstable).
  - Telescoped L1: within an interval, L1psum(s+1) = L1psum(s) + W31^T h2(s)
    + s*c0 accumulated in PSUM (W31 = DT*W3@W1), so only the first substep
    pays the 528-contraction and the PE never waits on a DVE round trip at
    step boundaries.
  - cn1 state feature order is permuted to [psi(512); y(16)] so the obs-step
    reset cn1 <- [y_t, cn0] is partition-aligned with cn0 (pure elementwise).
  - Sequential time loop (T=50 obs x 5 euler) fully unrolled.
"""

import os
from contextlib import ExitStack

import numpy as np
import ml_dtypes

Nc, ID, HID = 32, 16, 512
DT, N_STEPS, B, T = 0.05, 5, 256, 50
D0 = ID * Nc            # 512
DIN = D0 + ID           # 528
NCORES = 8
BL = B // NCORES        # 32 batch per core
DELTA = 5.0

USE_BF16 = os.environ.get("CNODE_BF16", "0") == "1"
USE_TELE = os.environ.get("CNODE_TELE", "1") == "1"
T_RUN = int(os.environ.get("CNODE_T_RUN", str(T)))  # dev knob; harness uses 50

_CACHE: dict = {}
LAST_RESULT = None


# ---------------------------------------------------------------- program ---
def _build(bf16: bool, t_run: int, tele: bool, reps: int = 1):
    import concourse.bass as bass
    from concourse import bacc, mybir, tile

    f32 = mybir.dt.float32
    dtw = mybir.dt.bfloat16 if bf16 else f32
    ADD = mybir.AluOpType.add
    SUB = mybir.AluOpType.subtract
    MUL = mybir.AluOpType.mult
    MAX = mybir.AluOpType.max

    nc = bacc.Bacc("TRN2", target_bir_lowering=False, debug=False,
                   num_devices=NCORES)

    w1 = nc.dram_tensor("w1", [128, 5 * 512], dtw, kind="ExternalInput").ap()
    w2 = nc.dram_tensor("w2", [128, 4 * 512], dtw, kind="ExternalInput").ap()
    w3 = nc.dram_tensor("w3", [128, 4 * 528], dtw, kind="ExternalInput").ap()
    bd = nc.dram_tensor("bd", [128, 128], f32, kind="ExternalInput").ap()
    cu = nc.dram_tensor("cu", [16 * N_STEPS, 512], f32, kind="ExternalInput").ap()
    if tele:
        w31 = nc.dram_tensor("w31", [128, 4 * 512], dtw,
                             kind="ExternalInput").ap()
    b1 = nc.dram_tensor("b1", [128, 4 * N_STEPS], f32,
                        kind="ExternalInput").ap()
    b2 = nc.dram_tensor("b2", [128, 4], f32, kind="ExternalInput").ap()
    b3 = nc.dram_tensor("b3", [128, 5], f32, kind="ExternalInput").ap()
    yt = nc.dram_tensor("yt", [16, 32 * t_run], f32, kind="ExternalInput").ap()
    mt = nc.dram_tensor("mt", [128, 128 * t_run], f32, kind="ExternalInput").ap()

    preds = nc.dram_tensor("preds", [16, 32 * t_run], f32, kind="ExternalOutput").ap()
    traj0 = nc.dram_tensor("traj0", [t_run, 128, 128], f32, kind="ExternalOutput").ap()
    fin1 = nc.dram_tensor("fin1", [128, 160], f32, kind="ExternalOutput").ap()

    with tile.TileContext(nc) as tc, ExitStack() as ctx:
        wp = ctx.enter_context(tc.tile_pool(name="weights", bufs=1))
        sp = ctx.enter_context(tc.tile_pool(name="state", bufs=1))
        ap_ = ctx.enter_context(tc.tile_pool(name="acts", bufs=2))
        pp = ctx.enter_context(tc.tile_pool(name="psum", bufs=2, space="PSUM"))

        W1t = wp.tile([128, 5 * 512], dtw)
        W2t = wp.tile([128, 4 * 512], dtw)
        W3t = wp.tile([128, 4 * 528], dtw)
        if tele:
            W31t = wp.tile([128, 4 * 512], dtw)
        BDt = wp.tile([128, 128], f32)
        CUt = wp.tile([16 * N_STEPS, 512], f32)
        b1t = wp.tile([128, 4 * N_STEPS], f32)
        b2t = wp.tile([128, 4], f32)
        b3t = wp.tile([128, 5], f32)
        Yt = wp.tile([16, 32 * t_run], f32)
        Mt = wp.tile([128, 128 * t_run], f32)
        predt = wp.tile([16, 32 * t_run], f32)

        dma_pairs = [(W1t, w1), (W2t, w2), (W3t, w3), (BDt, bd), (CUt, cu),
                     (b1t, b1), (b2t, b2), (b3t, b3), (Yt, yt), (Mt, mt)]
        if tele:
            dma_pairs.append((W31t, w31))
        for dst, src in dma_pairs:
            nc.sync.dma_start(dst[:], src[:])

        cn0 = sp.tile([128, 128], f32)
        cn1 = sp.tile([128, 160], f32)
        Ut = sp.tile([16 * N_STEPS, 32], f32)
        if bf16:
            cn1x = sp.tile([128, 160], dtw)
        else:
            cn1x = cn1

        def x_slice(k):
            # MLP input K-tile k (feature-major): psi blocks 0..3, then y.
            if k < 4:
                return cn1x[:, 32 * k:32 * k + 32]
            return cn1x[0:16, 128:160]

        def euler_step(p1, s, h2_prev, mid_cb=None):
            p2 = pp.tile([128, 128], f32, tag="p2")
            p3 = pp.tile([128, 160], f32, tag="p3")
            h1 = [ap_.tile([128, 32], dtw, tag=f"h1_{m}", name=f"h1_{m}")
                  for m in range(4)]
            h2 = [ap_.tile([128, 32], dtw, tag=f"h2_{m}", name=f"h2_{m}")
                  for m in range(4)]

            # capture u(s) = current y-part into the stacked-U tile (feeds the
            # folded cn0 update at interval end)
            nc.sync.dma_start(Ut[16 * s:16 * s + 16, :], cn1[0:16, 128:160])

            # L1: h1 = relu(L1psum + b1 + s*c0)
            # s==0: L1psum = W1p^T x (fresh, 528-contraction)
            # s>0 (tele): L1psum += W31^T h2_prev  (accumulate in PSUM)
            if tele and s > 0:
                for m in range(4):
                    for k in range(4):
                        nc.tensor.matmul(
                            p1[:, 32 * m:32 * m + 32],
                            W31t[:, k * 512 + m * 128:k * 512 + m * 128 + 128],
                            h2_prev[k][:, :],
                            start=False, stop=(k == 3), skip_group_check=True)
            else:
                for m in range(4):
                    for k in range(5):
                        kk = 128 if k < 4 else 16
                        # start=True clears has_written for the WHOLE bank, so
                        # in tele mode only the first MM of the interval may
                        # set it (later steps accumulate onto every m-tile).
                        st = (m == 0 and k == 0) if tele else (k == 0)
                        nc.tensor.matmul(
                            p1[:, 32 * m:32 * m + 32],
                            W1t[0:kk, k * 512 + m * 128:k * 512 + m * 128 + 128],
                            x_slice(k), start=st, stop=(k == 4),
                            skip_group_check=tele)
            bs = (s * 4) if tele else 0
            for m in range(4):
                nc.vector.tensor_scalar(
                    h1[m][:, :], p1[:, 32 * m:32 * m + 32],
                    b1t[:, bs + m:bs + m + 1], 0.0, ADD, MAX)

            if mid_cb is not None:
                mid_cb()

            # L2: h2 = relu(W2^T h1 + b2)   (k-outer: MM (m,k) only needs h1
            # block k, so the PE never waits on the last relu of L1)
            for k in range(4):
                for m in range(4):
                    nc.tensor.matmul(
                        p2[:, 32 * m:32 * m + 32],
                        W2t[:, k * 512 + m * 128:k * 512 + m * 128 + 128],
                        h1[k][:, :],
                        start=(k == 0 and m == 0), stop=(k == 3),
                        skip_group_check=True)
            for m in range(4):
                nc.vector.tensor_scalar(
                    h2[m][:, :], p2[:, 32 * m:32 * m + 32],
                    b2t[:, m:m + 1], 0.0, ADD, MAX)

            # L3: p3 = W3p'^T h2   (DT and output permutation folded in)
            for k in range(4):
                for m in range(5):
                    mm = 128 if m < 4 else 16
                    oc = (slice(0, 128), slice(32 * m, 32 * m + 32)) if m < 4 \
                        else (slice(0, 16), slice(128, 160))
                    nc.tensor.matmul(
                        p3[oc[0], oc[1]],
                        W3t[:, k * 528 + m * 128:k * 528 + m * 128 + mm],
                        h2[k][:, :],
                        start=(k == 0 and m == 0), stop=(k == 3),
                        skip_group_check=True)

            # cn1 += p3 + b3'   (bf16 shadow first: it gates next step's L1)
            for m in range(5):
                if m < 4:
                    po, co = slice(0, 128), slice(32 * m, 32 * m + 32)
                    bcol = b3t[:, m:m + 1]
                else:
                    po, co = slice(0, 16), slice(128, 160)
                    bcol = b3t[0:16, 4:5]
                if bf16 and not tele:
                    nc.vector.scalar_tensor_tensor(
                        cn1x[po, co], p3[po, co], bcol, cn1[po, co], ADD, ADD)
                nc.vector.scalar_tensor_tensor(
                    cn1[po, co], p3[po, co], bcol, cn1[po, co], ADD, ADD)
            return h2

        def cn0_interval_update():
            # cn0 <- G5-blockdiag^T cn0 + CU^T U   (whole obs interval folded)
            pc = pp.tile([128, 128], f32, tag="pc")
            nc.tensor.matmul(pc[:, 0:128], BDt[:, :], cn0[:, 0:128],
                             start=True, stop=False)
            for d in range(4):
                nc.tensor.matmul(pc[:, 32 * d:32 * d + 32],
                                 CUt[0:80, 128 * d:128 * d + 128],
                                 Ut[0:80, :],
                                 start=False, stop=(d == 3),
                                 skip_group_check=True)
            nc.vector.tensor_copy(cn0[:, :], pc[:, 0:128])

        def obs_update(t):
            # prediction = y-part of h_end (pre-mix)
            nc.vector.tensor_copy(predt[:, 32 * t:32 * t + 32],
                                  cn1[0:16, 128:160])
            # cn1 <- cn1 + m * (upd - cn1); upd = [cn0 (psi); y_t]
            t1 = ap_.tile([128, 128], f32, tag="mix1")
            t2 = ap_.tile([128, 128], f32, tag="mix2")
            nc.vector.tensor_tensor(t1[:, :], cn0[:, :], cn1[:, 0:128], SUB)
            nc.vector.tensor_tensor(t2[:, :], t1[:, :],
                                    Mt[:, 128 * t:128 * t + 128], MUL)
            if bf16:
                nc.vector.tensor_tensor(cn1x[:, 0:128], t2[:, :],
                                        cn1[:, 0:128], ADD)
            nc.vector.tensor_tensor(cn1[:, 0:128], t2[:, :], cn1[:, 0:128], ADD)

            t1y = ap_.tile([16, 32], f32, tag="mix1y")
            t2y = ap_.tile([16, 32], f32, tag="mix2y")
            nc.vector.tensor_tensor(t1y[:, :], Yt[0:16, 32 * t:32 * t + 32],
                                    cn1[0:16, 128:160], SUB)
            nc.vector.tensor_tensor(t2y[:, :], t1y[:, :],
                                    Mt[0:16, 128 * t:128 * t + 32], MUL)
            if bf16:
                nc.vector.tensor_tensor(cn1x[0:16, 128:160], t2y[:, :],
                                        cn1[0:16, 128:160], ADD)
            nc.vector.tensor_tensor(cn1[0:16, 128:160], t2y[:, :],
                                    cn1[0:16, 128:160], ADD)

            nc.sync.dma_start(traj0[t], cn0[:, :])
            if t == t_run - 1:
                nc.sync.dma_start(fin1[:], cn1[:, :])

        for _rep in range(reps):
            nc.vector.memset(cn0[:], 0.0)
            nc.vector.memset(cn1[:], 0.0)
            if bf16:
                nc.vector.memset(cn1x[:], 0.0)
            for t in range(t_run):
                p1 = pp.tile([128, 128], f32, tag="p1")
                h2_prev = None
                for s in range(N_STEPS):
                    if not tele and s > 0:
                        p1 = pp.tile([128, 128], f32, tag="p1")
                    # overlap the folded cn0 update (whose U dependency is
                    # complete once substep 4's capture lands) with the last
                    # substep's MLP instead of serializing it at interval end
                    cb = cn0_interval_update if s == N_STEPS - 1 else None
                    h2_prev = euler_step(p1, s, h2_prev, mid_cb=cb)
                obs_update(t)

        nc.sync.dma_start(preds[:], predt[:])

    nc.compile()
    return nc


# ------------------------------------------------------------- host packing ---
def _pack_shared(inputs, bf16: bool, tele: bool):
    f32 = np.float32
    wdt = ml_dtypes.bfloat16 if bf16 else f32
    W1 = np.asarray(inputs["W1"], f32)
    W2 = np.asarray(inputs["W2"], f32)
    W3 = np.asarray(inputs["W3"], f32)
    b1v = np.asarray(inputs["b1"], f32)
    b2v = np.asarray(inputs["b2"], f32)
    b3v = np.asarray(inputs["b3"], f32)
    A = np.asarray(inputs["A"], f32)
    Bv = np.asarray(inputs["Bv"], f32)

    perm = np.r_[ID:DIN, 0:ID]             # cn1 features -> [psi; y]
    W1p = W1[perm, :]                      # [528, 512]
    W3p = (DT * W3)[:, perm]               # [512, 528]
    b3p = (DT * b3v)[perm]

    w1sb = np.zeros((128, 5 * 512), f32)
    for k in range(5):
        kk = 128 if k < 4 else 16
        w1sb[0:kk, k * 512:(k + 1) * 512] = W1p[k * 128:k * 128 + kk, :]
    w2sb = np.zeros((128, 4 * 512), f32)
    for k in range(4):
        w2sb[:, k * 512:(k + 1) * 512] = W2[k * 128:(k + 1) * 128, :]
    w3sb = np.zeros((128, 4 * 528), f32)
    for k in range(4):
        w3sb[:, k * 528:(k + 1) * 528] = W3p[k * 128:(k + 1) * 128, :]

    # Folded per-interval cn0 map: c(5) = G^5 c(0) + sum_s G^(4-s) DT*Bv u(s)
    G = np.eye(Nc, dtype=np.float64) + np.float64(DT) * A.astype(np.float64)
    G5 = np.linalg.matrix_power(G, N_STEPS)
    bdsb = np.kron(np.eye(4), G5.T).astype(f32)                    # [128,128]
    dtbv = np.float64(DT) * Bv.astype(np.float64)
    cusb = np.zeros((16 * N_STEPS, 512), np.float64)
    for s in range(N_STEPS):
        v = np.linalg.matrix_power(G, N_STEPS - 1 - s) @ dtbv      # [32]
        for i in range(16):
            base = 128 * (i // 4) + 32 * (i % 4)
            cusb[16 * s + i, base:base + 32] = v
    cusb = cusb.astype(f32)

    b2sb = b2v.reshape(4, 128).T.copy()
    b3sb = np.zeros((128, 5), f32)
    b3sb[:, 0:4] = b3p[0:512].reshape(4, 128).T
    b3sb[0:16, 4] = b3p[512:528]

    c0 = DT * (W1.T @ b3v)                 # telescoped bias increment [512]
    b1sb = np.zeros((128, 4 * N_STEPS), f32)
    for s in range(N_STEPS):
        b1sb[:, 4 * s:4 * s + 4] = (b1v + s * c0).reshape(4, 128).T

    out = {"w1": w1sb.astype(wdt), "w2": w2sb.astype(wdt),
           "w3": w3sb.astype(wdt), "bd": bdsb, "cu": cusb,
           "b1": b1sb, "b2": b2sb, "b3": b3sb}
    if tele:
        W31 = (DT * (W3 @ W1)).astype(f32)  # [512, 512]
        w31sb = np.zeros((128, 4 * 512), f32)
        for k in range(4):
            w31sb[:, k * 512:(k + 1) * 512] = W31[k * 128:(k + 1) * 128, :]
        out["w31"] = w31sb.astype(wdt)
    return out


def _pack_core(inputs, c: int, t_run: int):
    f32 = np.float32
    Y = np.asarray(inputs["Y"], f32)[c * BL:(c + 1) * BL]       # [32, T, 16]
    mask = np.asarray(inputs["mask"], f32)[c * BL:(c + 1) * BL]  # [32, T]
    ysb = Y[:, 0:t_run, :].transpose(2, 1, 0).reshape(16, t_run * 32).copy()
    mtb = mask[:, 0:t_run].T                                    # [t, b]
    mrow = np.tile(mtb, (1, 4)).reshape(1, t_run * 128)
    mtsb = np.broadcast_to(mrow, (128, t_run * 128)).copy()
    return {"yt": ysb, "mt": mtsb}


def kernel(**inputs):
    from concourse.bass_utils import run_bass_kernel_spmd

    bf16, t_run, tele = USE_BF16, T_RUN, USE_TELE
    key = (bf16, t_run, tele)
    if key not in _CACHE:
        _CACHE[key] = _build(bf16, t_run, tele)
    nc = _CACHE[key]

    shared = _pack_shared(inputs, bf16, tele)
    in_maps = [dict(shared, **_pack_core(inputs, c, t_run))
               for c in range(NCORES)]

    res = run_bass_kernel_spmd(nc, in_maps, core_ids=list(range(NCORES)))
    global LAST_RESULT
    LAST_RESULT = res
    outs = res.results

    f32 = np.float32
    mask = np.asarray(inputs["mask"], f32)[:, 0:t_run]
    y_preds = np.zeros((B, t_run, ID), f32)
    h_fin = np.zeros((B, 2 * D0 + ID), f32)
    cn0_traj = np.zeros((t_run, B, D0), f32)
    for c in range(NCORES):
        o = outs[c]
        sl = slice(c * BL, (c + 1) * BL)
        y_preds[sl] = np.asarray(o["preds"], f32).reshape(
            16, t_run, 32).transpose(2, 1, 0)
        tr = np.asarray(o["traj0"], f32).reshape(
            t_run, 128, 4, 32).transpose(0, 3, 2, 1).reshape(t_run, 32, D0)
        cn0_traj[:, sl, :] = tr
        f1 = np.asarray(o["fin1"], f32)
        psi = f1[:, 0:128].reshape(128, 4, 32).transpose(2, 1, 0).reshape(32, D0)
        yfin = f1[0:16, 128:160].T
        h_fin[sl, 0:D0] = tr[t_run - 1]
        h_fin[sl, D0:D0 + ID] = yfin
        h_fin[sl, D0 + ID:] = psi

    any_t = np.nonzero(mask.any(axis=0))[0]
    if len(any_t):
        last_h_cn = cn0_traj[any_t.max()].copy()
    else:
        last_h_cn = np.zeros((B, D0), f32)

    times = np.asarray(inputs["times"], f32)
    return y_preds, y_preds.copy(), times, last_h_cn, h_fin


# revision 44
# speedup vs baseline: 1.0859x; 1.0269x over previous
"""Trainium2 Bass kernel for the CNODE (HiPPO continuous-time ODE) model.

Strategy (8 NeuronCores, pure data parallel over batch B=256 -> 32/core):
  - All state kept feature-major in SBUF: partition = feature (mod 128),
    free dim = (feature_block, batch).
  - MLP layers are weight-stationary matmuls: lhsT = weight tile [K,128],
    rhs = activations [K, 32].  No transposes anywhere.
  - cn0 (HiPPO coefficients) update folded per obs interval on the host:
    c(5) = G^5 c(0) + sum_s G^(4-s)*DT*Bv u(s) with G = I + DT*A, so the
    5 euler substeps cost one kron(I4, G5^T) matmul (N=128) plus 4 stacked
    forcing matmuls (K=80) per interval.  This path stays fp32 (bf16 here
    is catastrophically unstable).
  - Telescoped L1: within an interval, L1psum(s+1) = L1psum(s) + W31^T h2(s)
    + s*c0 accumulated in PSUM (W31 = DT*W3@W1), so only the first substep
    pays the 528-contraction and the PE never waits on a DVE round trip at
    step boundaries.
  - cn1 state feature order is permuted to [psi(512); y(16)] so the obs-step
    reset cn1 <- [y_t, cn0] is partition-aligned with cn0 (pure elementwise).
  - Sequential time loop (T=50 obs x 5 euler) fully unrolled.
"""

import os
from contextlib import ExitStack

import numpy as np
import ml_dtypes

Nc, ID, HID = 32, 16, 512
DT, N_STEPS, B, T = 0.05, 5, 256, 50
D0 = ID * Nc            # 512
DIN = D0 + ID           # 528
NCORES = 8
BL = B // NCORES        # 32 batch per core
DELTA = 5.0

USE_BF16 = os.environ.get("CNODE_BF16", "0") == "1"
USE_TELE = os.environ.get("CNODE_TELE", "1") == "1"
T_RUN = int(os.environ.get("CNODE_T_RUN", str(T)))  # dev knob; harness uses 50

_CACHE: dict = {}
LAST_RESULT = None


# ---------------------------------------------------------------- program ---
def _build(bf16: bool, t_run: int, tele: bool, reps: int = 1):
    import concourse.bass as bass
    from concourse import bacc, mybir, tile

    f32 = mybir.dt.float32
    dtw = mybir.dt.bfloat16 if bf16 else f32
    ADD = mybir.AluOpType.add
    SUB = mybir.AluOpType.subtract
    MUL = mybir.AluOpType.mult
    MAX = mybir.AluOpType.max

    nc = bacc.Bacc("TRN2", target_bir_lowering=False, debug=False,
                   num_devices=NCORES)

    w1 = nc.dram_tensor("w1", [128, 5 * 512], dtw, kind="ExternalInput").ap()
    w2 = nc.dram_tensor("w2", [128, 4 * 512], dtw, kind="ExternalInput").ap()
    w3 = nc.dram_tensor("w3", [128, 4 * 528], dtw, kind="ExternalInput").ap()
    bd = nc.dram_tensor("bd", [128, 128], f32, kind="ExternalInput").ap()
    cu = nc.dram_tensor("cu", [16 * N_STEPS, 512], f32, kind="ExternalInput").ap()
    if tele:
        w31 = nc.dram_tensor("w31", [128, 4 * 512], dtw,
                             kind="ExternalInput").ap()
    b1 = nc.dram_tensor("b1", [128, 4 * N_STEPS], f32,
                        kind="ExternalInput").ap()
    b2 = nc.dram_tensor("b2", [128, 4], f32, kind="ExternalInput").ap()
    b3 = nc.dram_tensor("b3", [128, 5], f32, kind="ExternalInput").ap()
    b3r = nc.dram_tensor("b3r", [128, 128], f32, kind="ExternalInput").ap()
    yt = nc.dram_tensor("yt", [16, 32 * t_run], f32, kind="ExternalInput").ap()
    mt = nc.dram_tensor("mt", [128, 128 * t_run], f32, kind="ExternalInput").ap()

    preds = nc.dram_tensor("preds", [16, 32 * t_run], f32, kind="ExternalOutput").ap()
    traj0 = nc.dram_tensor("traj0", [t_run, 128, 128], f32, kind="ExternalOutput").ap()
    fin1 = nc.dram_tensor("fin1", [128, 160], f32, kind="ExternalOutput").ap()

    with tile.TileContext(nc) as tc, ExitStack() as ctx:
        wp = ctx.enter_context(tc.tile_pool(name="weights", bufs=1))
        sp = ctx.enter_context(tc.tile_pool(name="state", bufs=1))
        ap_ = ctx.enter_context(tc.tile_pool(name="acts", bufs=2))
        pp = ctx.enter_context(tc.tile_pool(name="psum", bufs=2, space="PSUM"))

        W1t = wp.tile([128, 5 * 512], dtw)
        W2t = wp.tile([128, 4 * 512], dtw)
        W3t = wp.tile([128, 4 * 528], dtw)
        if tele:
            W31t = wp.tile([128, 4 * 512], dtw)
        BDt = wp.tile([128, 128], f32)
        CUt = wp.tile([16 * N_STEPS, 512], f32)
        b1t = wp.tile([128, 4 * N_STEPS], f32)
        b2t = wp.tile([128, 4], f32)
        b3t = wp.tile([128, 5], f32)
        b3rt = wp.tile([128, 128], f32)
        Yt = wp.tile([16, 32 * t_run], f32)
        Mt = wp.tile([128, 128 * t_run], f32)
        predt = wp.tile([16, 32 * t_run], f32)

        dma_pairs = [(W1t, w1), (W2t, w2), (W3t, w3), (BDt, bd), (CUt, cu),
                     (b1t, b1), (b2t, b2), (b3t, b3), (b3rt, b3r),
                     (Yt, yt), (Mt, mt)]
        if tele:
            dma_pairs.append((W31t, w31))
        for dst, src in dma_pairs:
            nc.sync.dma_start(dst[:], src[:])

        cn0 = sp.tile([128, 128], f32)
        cn1 = sp.tile([128, 160], f32)
        Ut = sp.tile([16 * N_STEPS, 32], f32)
        if bf16:
            cn1x = sp.tile([128, 160], dtw)
        else:
            cn1x = cn1

        def x_slice(k):
            # MLP input K-tile k (feature-major): psi blocks 0..3, then y.
            if k < 4:
                return cn1x[:, 32 * k:32 * k + 32]
            return cn1x[0:16, 128:160]

        def euler_step(p1, s, h2_prev, mid_cb=None):
            p2 = pp.tile([128, 128], f32, tag="p2")
            p3 = pp.tile([128, 160], f32, tag="p3")
            h1 = [ap_.tile([128, 32], dtw, tag=f"h1_{m}", name=f"h1_{m}")
                  for m in range(4)]
            h2 = [ap_.tile([128, 32], dtw, tag=f"h2_{m}", name=f"h2_{m}")
                  for m in range(4)]

            # capture u(s) = current y-part into the stacked-U tile (feeds the
            # folded cn0 update at interval end)
            nc.sync.dma_start(Ut[16 * s:16 * s + 16, :], cn1[0:16, 128:160])

            # L1: h1 = relu(L1psum + b1 + s*c0)
            # s==0: L1psum = W1p^T x (fresh, 528-contraction)
            # s>0 (tele): L1psum += W31^T h2_prev  (accumulate in PSUM)
            if tele and s > 0:
                for m in range(4):
                    for k in range(4):
                        nc.tensor.matmul(
                            p1[:, 32 * m:32 * m + 32],
                            W31t[:, k * 512 + m * 128:k * 512 + m * 128 + 128],
                            h2_prev[k][:, :],
                            start=False, stop=(k == 3), skip_group_check=True)
            else:
                for m in range(4):
                    for k in range(5):
                        kk = 128 if k < 4 else 16
                        # start=True clears has_written for the WHOLE bank, so
                        # in tele mode only the first MM of the interval may
                        # set it (later steps accumulate onto every m-tile).
                        st = (m == 0 and k == 0) if tele else (k == 0)
                        nc.tensor.matmul(
                            p1[:, 32 * m:32 * m + 32],
                            W1t[0:kk, k * 512 + m * 128:k * 512 + m * 128 + 128],
                            x_slice(k), start=st, stop=(k == 4),
                            skip_group_check=tele)
            bs = (s * 4) if tele else 0
            for m in range(4):
                nc.vector.tensor_scalar(
                    h1[m][:, :], p1[:, 32 * m:32 * m + 32],
                    b1t[:, bs + m:bs + m + 1], 0.0, ADD, MAX)

            if mid_cb is not None:
                mid_cb()

            # L2: h2 = relu(W2^T h1 + b2)   (k-outer: MM (m,k) only needs h1
            # block k, so the PE never waits on the last relu of L1)
            for k in range(4):
                for m in range(4):
                    nc.tensor.matmul(
                        p2[:, 32 * m:32 * m + 32],
                        W2t[:, k * 512 + m * 128:k * 512 + m * 128 + 128],
                        h1[k][:, :],
                        start=(k == 0 and m == 0), stop=(k == 3),
                        skip_group_check=True)
            for m in range(4):
                nc.vector.tensor_scalar(
                    h2[m][:, :], p2[:, 32 * m:32 * m + 32],
                    b2t[:, m:m + 1], 0.0, ADD, MAX)

            # L3: p3 = W3p'^T h2   (DT and output permutation folded in)
            for k in range(4):
                for m in range(5):
                    mm = 128 if m < 4 else 16
                    oc = (slice(0, 128), slice(32 * m, 32 * m + 32)) if m < 4 \
                        else (slice(0, 16), slice(128, 160))
                    nc.tensor.matmul(
                        p3[oc[0], oc[1]],
                        W3t[:, k * 528 + m * 128:k * 528 + m * 128 + mm],
                        h2[k][:, :],
                        start=(k == 0 and m == 0), stop=(k == 3),
                        skip_group_check=True)

            # cn1 += p3 + b3'  — psi part as two wide ops (same association:
            # (p3 + b3) + cn1), y part via stt
            if bf16 and not tele:
                for m in range(5):
                    if m < 4:
                        po, co = slice(0, 128), slice(32 * m, 32 * m + 32)
                        bcol = b3t[:, m:m + 1]
                    else:
                        po, co = slice(0, 16), slice(128, 160)
                        bcol = b3t[0:16, 4:5]
                    nc.vector.scalar_tensor_tensor(
                        cn1x[po, co], p3[po, co], bcol, cn1[po, co], ADD, ADD)
                    nc.vector.scalar_tensor_tensor(
                        cn1[po, co], p3[po, co], bcol, cn1[po, co], ADD, ADD)
            else:
                e3 = ap_.tile([128, 128], f32, tag="e3")
                nc.vector.tensor_tensor(e3[:, :], p3[:, 0:128], b3rt[:, :], ADD)
                nc.vector.tensor_tensor(cn1[:, 0:128], e3[:, :],
                                        cn1[:, 0:128], ADD)
                nc.vector.scalar_tensor_tensor(
                    cn1[0:16, 128:160], p3[0:16, 128:160], b3t[0:16, 4:5],
                    cn1[0:16, 128:160], ADD, ADD)
            return h2

        def cn0_interval_update():
            # cn0 <- G5-blockdiag^T cn0 + CU^T U   (whole obs interval folded)
            pc = pp.tile([128, 128], f32, tag="pc")
            nc.tensor.matmul(pc[:, 0:128], BDt[:, :], cn0[:, 0:128],
                             start=True, stop=False)
            for d in range(4):
                nc.tensor.matmul(pc[:, 32 * d:32 * d + 32],
                                 CUt[0:80, 128 * d:128 * d + 128],
                                 Ut[0:80, :],
                                 start=False, stop=(d == 3),
                                 skip_group_check=True)
            nc.vector.tensor_copy(cn0[:, :], pc[:, 0:128])

        def obs_update(t):
            # psi-mix first: it gates the next interval's L1 k=0..3 matmuls
            # cn1 <- cn1 + m * (upd - cn1); upd = [cn0 (psi); y_t]
            t1 = ap_.tile([128, 128], f32, tag="mix1")
            t2 = ap_.tile([128, 128], f32, tag="mix2")
            nc.vector.tensor_tensor(t1[:, :], cn0[:, :], cn1[:, 0:128], SUB)
            nc.vector.tensor_tensor(t2[:, :], t1[:, :],
                                    Mt[:, 128 * t:128 * t + 128], MUL)
            if bf16:
                nc.vector.tensor_tensor(cn1x[:, 0:128], t2[:, :],
                                        cn1[:, 0:128], ADD)
            nc.vector.tensor_tensor(cn1[:, 0:128], t2[:, :], cn1[:, 0:128], ADD)

            # prediction = y-part of h_end (pre-mix)
            nc.vector.tensor_copy(predt[:, 32 * t:32 * t + 32],
                                  cn1[0:16, 128:160])
            t1y = ap_.tile([16, 32], f32, tag="mix1y")
            t2y = ap_.tile([16, 32], f32, tag="mix2y")
            nc.vector.tensor_tensor(t1y[:, :], Yt[0:16, 32 * t:32 * t + 32],
                                    cn1[0:16, 128:160], SUB)
            nc.vector.tensor_tensor(t2y[:, :], t1y[:, :],
                                    Mt[0:16, 128 * t:128 * t + 32], MUL)
            if bf16:
                nc.vector.tensor_tensor(cn1x[0:16, 128:160], t2y[:, :],
                                        cn1[0:16, 128:160], ADD)
            nc.vector.tensor_tensor(cn1[0:16, 128:160], t2y[:, :],
                                    cn1[0:16, 128:160], ADD)

            nc.sync.dma_start(traj0[t], cn0[:, :])
            if t == t_run - 1:
                nc.sync.dma_start(fin1[:], cn1[:, :])

        for _rep in range(reps):
            nc.vector.memset(cn0[:], 0.0)
            nc.vector.memset(cn1[:], 0.0)
            if bf16:
                nc.vector.memset(cn1x[:], 0.0)
            for t in range(t_run):
                p1 = pp.tile([128, 128], f32, tag="p1")
                h2_prev = None
                for s in range(N_STEPS):
                    if not tele and s > 0:
                        p1 = pp.tile([128, 128], f32, tag="p1")
                    # overlap the folded cn0 update (whose U dependency is
                    # complete once substep 4's capture lands) with the last
                    # substep's MLP instead of serializing it at interval end
                    cb = cn0_interval_update if s == N_STEPS - 1 else None
                    h2_prev = euler_step(p1, s, h2_prev, mid_cb=cb)
                obs_update(t)

        nc.sync.dma_start(preds[:], predt[:])

    nc.compile()
    return nc


# ------------------------------------------------------------- host packing ---
def _pack_shared(inputs, bf16: bool, tele: bool):
    f32 = np.float32
    wdt = ml_dtypes.bfloat16 if bf16 else f32
    W1 = np.asarray(inputs["W1"], f32)
    W2 = np.asarray(inputs["W2"], f32)
    W3 = np.asarray(inputs["W3"], f32)
    b1v = np.asarray(inputs["b1"], f32)
    b2v = np.asarray(inputs["b2"], f32)
    b3v = np.asarray(inputs["b3"], f32)
    A = np.asarray(inputs["A"], f32)
    Bv = np.asarray(inputs["Bv"], f32)

    perm = np.r_[ID:DIN, 0:ID]             # cn1 features -> [psi; y]
    W1p = W1[perm, :]                      # [528, 512]
    W3p = (DT * W3)[:, perm]               # [512, 528]
    b3p = (DT * b3v)[perm]

    w1sb = np.zeros((128, 5 * 512), f32)
    for k in range(5):
        kk = 128 if k < 4 else 16
        w1sb[0:kk, k * 512:(k + 1) * 512] = W1p[k * 128:k * 128 + kk, :]
    w2sb = np.zeros((128, 4 * 512), f32)
    for k in range(4):
        w2sb[:, k * 512:(k + 1) * 512] = W2[k * 128:(k + 1) * 128, :]
    w3sb = np.zeros((128, 4 * 528), f32)
    for k in range(4):
        w3sb[:, k * 528:(k + 1) * 528] = W3p[k * 128:(k + 1) * 128, :]

    # Folded per-interval cn0 map: c(5) = G^5 c(0) + sum_s G^(4-s) DT*Bv u(s)
    G = np.eye(Nc, dtype=np.float64) + np.float64(DT) * A.astype(np.float64)
    G5 = np.linalg.matrix_power(G, N_STEPS)
    bdsb = np.kron(np.eye(4), G5.T).astype(f32)                    # [128,128]
    dtbv = np.float64(DT) * Bv.astype(np.float64)
    cusb = np.zeros((16 * N_STEPS, 512), np.float64)
    for s in range(N_STEPS):
        v = np.linalg.matrix_power(G, N_STEPS - 1 - s) @ dtbv      # [32]
        for i in range(16):
            base = 128 * (i // 4) + 32 * (i % 4)
            cusb[16 * s + i, base:base + 32] = v
    cusb = cusb.astype(f32)

    b2sb = b2v.reshape(4, 128).T.copy()
    b3sb = np.zeros((128, 5), f32)
    b3sb[:, 0:4] = b3p[0:512].reshape(4, 128).T
    b3sb[0:16, 4] = b3p[512:528]
    b3rsb = np.repeat(b3sb[:, 0:4], 32, axis=1).copy()  # [128, 128] per-m rep

    c0 = DT * (W1.T @ b3v)                 # telescoped bias increment [512]
    b1sb = np.zeros((128, 4 * N_STEPS), f32)
    for s in range(N_STEPS):
        b1sb[:, 4 * s:4 * s + 4] = (b1v + s * c0).reshape(4, 128).T

    out = {"w1": w1sb.astype(wdt), "w2": w2sb.astype(wdt),
           "w3": w3sb.astype(wdt), "bd": bdsb, "cu": cusb,
           "b1": b1sb, "b2": b2sb, "b3": b3sb, "b3r": b3rsb}
    if tele:
        W31 = (DT * (W3 @ W1)).astype(f32)  # [512, 512]
        w31sb = np.zeros((128, 4 * 512), f32)
        for k in range(4):
            w31sb[:, k * 512:(k + 1) * 512] = W31[k * 128:(k + 1) * 128, :]
        out["w31"] = w31sb.astype(wdt)
    return out


def _pack_core(inputs, c: int, t_run: int):
    f32 = np.float32
    Y = np.asarray(inputs["Y"], f32)[c * BL:(c + 1) * BL]       # [32, T, 16]
    mask = np.asarray(inputs["mask"], f32)[c * BL:(c + 1) * BL]  # [32, T]
    ysb = Y[:, 0:t_run, :].transpose(2, 1, 0).reshape(16, t_run * 32).copy()
    mtb = mask[:, 0:t_run].T                                    # [t, b]
    mrow = np.tile(mtb, (1, 4)).reshape(1, t_run * 128)
    mtsb = np.broadcast_to(mrow, (128, t_run * 128)).copy()
    return {"yt": ysb, "mt": mtsb}


def kernel(**inputs):
    from concourse.bass_utils import run_bass_kernel_spmd

    bf16, t_run, tele = USE_BF16, T_RUN, USE_TELE
    key = (bf16, t_run, tele)
    if key not in _CACHE:
        _CACHE[key] = _build(bf16, t_run, tele)
    nc = _CACHE[key]

    shared = _pack_shared(inputs, bf16, tele)
    in_maps = [dict(shared, **_pack_core(inputs, c, t_run))
               for c in range(NCORES)]

    res = run_bass_kernel_spmd(nc, in_maps, core_ids=list(range(NCORES)))
    global LAST_RESULT
    LAST_RESULT = res
    outs = res.results

    f32 = np.float32
    mask = np.asarray(inputs["mask"], f32)[:, 0:t_run]
    y_preds = np.zeros((B, t_run, ID), f32)
    h_fin = np.zeros((B, 2 * D0 + ID), f32)
    cn0_traj = np.zeros((t_run, B, D0), f32)
    for c in range(NCORES):
        o = outs[c]
        sl = slice(c * BL, (c + 1) * BL)
        y_preds[sl] = np.asarray(o["preds"], f32).reshape(
            16, t_run, 32).transpose(2, 1, 0)
        tr = np.asarray(o["traj0"], f32).reshape(
            t_run, 128, 4, 32).transpose(0, 3, 2, 1).reshape(t_run, 32, D0)
        cn0_traj[:, sl, :] = tr
        f1 = np.asarray(o["fin1"], f32)
        psi = f1[:, 0:128].reshape(128, 4, 32).transpose(2, 1, 0).reshape(32, D0)
        yfin = f1[0:16, 128:160].T
        h_fin[sl, 0:D0] = tr[t_run - 1]
        h_fin[sl, D0:D0 + ID] = yfin
        h_fin[sl, D0 + ID:] = psi

    any_t = np.nonzero(mask.any(axis=0))[0]
    if len(any_t):
        last_h_cn = cn0_traj[any_t.max()].copy()
    else:
        last_h_cn = np.zeros((B, D0), f32)

    times = np.asarray(inputs["times"], f32)
    return y_preds, y_preds.copy(), times, last_h_cn, h_fin
